# revision 1
# baseline (speedup 1.0000x reference)
"""CascadePredictor Trainium2 kernel: 2-layer GCN encode + collapsed MHA edge decode.

Distribution: 8-core SPMD, node-partitioned aggregation (load-balanced permuted
blocks), AllGather between layers, edge-parallel decode.

Algorithm (validated vs reference, numpy prototype):
  dinv[n] = 1/sqrt(indeg+1) (0 for pad nodes)
  hxd = (x @ W1 + b1) * dinv                       (bf16 table)
  h   = relu(dinv * (sum_{e: dst=d} hxd[src] + hxd[d]))
  hw2d= (h @ W2 + b2) * dinv                       (bf16 table, AllGather)
  z   = dinv * (sum hw2d[src] + hw2d[d])
  Tq  = [z@WqT*s | l0 | s0],  Tk = [z@WkT | s1]    (bf16 tables, AllGather)
  out = sigmoid(sum_h s0 + sigmoid(l1-l0)*(s1-s0) + bsum)   l1 = Q'[sp].K[dp]
"""
import sys
import numpy as np

for p in ("/opt/trn_rl_repo",):
    if p not in sys.path:
        sys.path.insert(0, p)

import ml_dtypes
import concourse.bass as bass
import concourse.bacc as bacc
import concourse.tile as tile
import concourse.mybir as mybir

bf16 = ml_dtypes.bfloat16
F32 = mybir.dt.float32
BF = mybir.dt.bfloat16
I32 = mybir.dt.int32

NCORES = 8
P = 128
HIDDEN = 256
NH, HD = 4, 64


# ----------------------------------------------------------------------------
# host-side preprocessing
# ----------------------------------------------------------------------------
def build_host_data(x, edge_index, edge_index_pred,
                    W1, b1, W2, b2, in_proj_w, in_proj_b, out_proj_w, out_proj_b):
    N = x.shape[0]
    src = np.asarray(edge_index[0], np.int64)
    dst = np.asarray(edge_index[1], np.int64)
    sp = np.asarray(edge_index_pred[0], np.int64)
    dp = np.asarray(edge_index_pred[1], np.int64)
    E = src.shape[0]
    EP = sp.shape[0]

    NBLK = -(-N // P)                      # blocks over real nodes
    NBLK = -(-NBLK // NCORES) * NCORES     # multiple of NCORES
    NPAD = NBLK * P
    NBC = NBLK // NCORES                   # blocks per core

    deg = np.bincount(dst, minlength=N).astype(np.float64) + 1.0
    dinv = np.zeros(NPAD, np.float32)
    dinv[:N] = (1.0 / np.sqrt(deg)).astype(np.float32)

    # --- load-balanced permutation: snake-assign nodes (sorted by indeg desc)
    indeg = (deg - 1.0).astype(np.int64)
    order = np.argsort(-indeg, kind="stable")
    snake = np.empty(N, np.int64)          # block id per sorted position
    pos = np.arange(N)
    rnd, off = pos // NBLK, pos % NBLK
    fwd = (rnd % 2) == 0
    snake[fwd] = off[fwd]
    snake[~fwd] = NBLK - 1 - off[~fwd]
    blk_of = np.empty(NPAD, np.int64)      # node -> block
    blk_of[order] = snake[:N]
    # pad nodes fill remaining slots
    slot_of = np.empty(NPAD, np.int64)
    # count real nodes per block, assign slots in order of appearance
    perm_sorted = np.argsort(blk_of[:N] * (NPAD + 1) + np.arange(N), kind="stable")
    # simpler: for each block, members = real nodes in it (<=P), then pads
    counts = np.bincount(blk_of[:N], minlength=NBLK)
    assert counts.max() <= P
    # stable order of real nodes by block
    o2 = np.argsort(blk_of[:N], kind="stable")
    within = np.arange(N) - np.repeat(np.concatenate([[0], np.cumsum(counts)[:-1]]), counts)
    slot_of[o2] = within
    # pads: fill blocks with free slots
    free_blocks = np.repeat(np.arange(NBLK), P - counts)
    pad_ids = np.arange(N, NPAD)
    blk_of[pad_ids] = free_blocks[: NPAD - N]
    pad_within = []
    fc = counts.copy()
    for b in free_blocks[: NPAD - N]:
        pad_within.append(fc[b])
        fc[b] += 1
    slot_of[pad_ids] = np.array(pad_within, np.int64) if len(pad_within) else np.zeros(0, np.int64)
    perm = blk_of * P + slot_of            # node -> permuted row
    assert np.array_equal(np.sort(perm), np.arange(NPAD))

    dinv_perm = np.zeros(NPAD, np.float32)
    dinv_perm[perm] = dinv                 # dinv for permuted rows (pads are 0)

    # --- edge grids: per block, edges grouped, padded; + self tile last
    pdst = perm[dst]
    psrc = perm[src]
    eblk = pdst // P
    eloc = pdst % P
    ecnt = np.bincount(eblk, minlength=NBLK)
    TE = int(-(-ecnt.max() // P))          # edge tiles per block
    T = TE                                 # self-loop handled via shard DMA
    eord = np.argsort(eblk, kind="stable")
    starts = np.concatenate([[0], np.cumsum(ecnt)[:-1]])
    epos = np.arange(E) - np.repeat(starts, ecnt)
    gsrc = np.zeros((NBLK, P, T), np.int32)
    dstloc = np.full((NBLK, P, T), -1.0, np.float32)
    b_, p_, t_ = eblk[eord], (epos % P), (epos // P)
    gsrc[b_, p_, t_] = psrc[eord].astype(np.int32)
    dstloc[b_, p_, t_] = eloc[eord].astype(np.float32)

    # per-core resident layouts [P, NBC*T]
    g4 = gsrc.reshape(NCORES, NBC, P, T)
    d4 = dstloc.reshape(NCORES, NBC, P, T)
    gsrc_core = [np.ascontiguousarray(g4[c].transpose(1, 0, 2).reshape(P, NBC * T)) for c in range(NCORES)]
    dstloc_core = [np.ascontiguousarray(d4[c].transpose(1, 0, 2).reshape(P, NBC * T)).astype(bf16) for c in range(NCORES)]

    # --- decode edge split: sp-sorted tiles whose sp-panels fit a sliding
    # window [phi(t), phi(t)+KW), so the Q side streams from sequential panels.
    KW = 3
    EPC_raw = -(-EP // NCORES)
    core_psp, core_pdp, core_orig = [], [], []
    for c in range(NCORES):
        lo, hi = c * EPC_raw, min((c + 1) * EPC_raw, EP)
        ps_ = perm[sp[lo:hi]]
        od = np.argsort(ps_, kind="stable")
        core_psp.append(ps_[od])
        core_pdp.append(perm[dp[lo:hi]][od])
        core_orig.append(np.arange(lo, hi)[od])

    def try_pack(pj, NDT2):
        nslots = NDT2 * P
        slot_edge = np.full(nslots, -1, np.int64)
        t, slot = 0, 0
        for i, j in enumerate(pj):
            while True:
                if t >= NDT2:
                    return None
                phi = (t * NBLK) // NDT2
                if j < phi:
                    return None
                if j >= phi + KW:
                    t += 1
                    slot = 0
                    continue
                break
            slot_edge[t * P + slot] = i
            slot += 1
            if slot == P:
                t += 1
                slot = 0
        return slot_edge

    base = -(-EPC_raw // P)
    base = -(-base // 4) * 4
    NDT = None
    for cand in range(base, base + 64, 4):
        packs = [try_pack(core_psp[c] // P, cand) for c in range(NCORES)]
        if all(pk is not None for pk in packs):
            NDT = cand
            break
    assert NDT is not None, "decode window packing failed"
    EPC = NDT * P
    sploc_core, dpi, invmap = [], [], []
    for c in range(NCORES):
        pk = packs[c]
        valid = pk >= 0
        psp_s = np.where(valid, core_psp[c][np.maximum(pk, 0)], -1)
        dp_s = np.where(valid, core_pdp[c][np.maximum(pk, 0)], 0)
        inv = np.where(valid, core_orig[c][np.maximum(pk, 0)], -1)
        # sploc[t, slot, k]: row within panel phi(t)+k, else -1
        sl = np.full((NDT, P, KW), -1.0, np.float32)
        tt = np.arange(NDT)
        phis = (tt * NBLK) // NDT
        pj = psp_s.reshape(NDT, P) // P
        pr = psp_s.reshape(NDT, P) % P
        for k in range(KW):
            hit = (pj == (phis[:, None] + k)) & (psp_s.reshape(NDT, P) >= 0)
            sl[:, :, k] = np.where(hit, pr, -1).astype(np.float32)
        sploc_core.append(np.ascontiguousarray(
            sl.transpose(1, 0, 2).reshape(P, NDT * KW)).astype(bf16))
        if c == 0:
            active = (sl >= 0).any(axis=1)
        else:
            active |= (sl >= 0).any(axis=1)
        dpi.append(np.ascontiguousarray(
            dp_s.reshape(NDT, P).T).astype(np.int32))
        invmap.append(inv)

    # --- dense weights / tables
    xp = np.zeros((NPAD, x.shape[1]), np.float32)
    xp[perm[:N]] = np.asarray(x, np.float32)[:N]  # permuted rows
    xT = np.ascontiguousarray(xp.T).astype(bf16)  # [IN_CH, NPAD]

    dinv_cols = np.ascontiguousarray(dinv_perm.reshape(NBLK, P).T)  # [P, NBLK] f32

    H = HIDDEN
    Wq = in_proj_w[0:H]; Wk = in_proj_w[H:2 * H]; Wv = in_proj_w[2 * H:3 * H]
    bq = in_proj_b[0:H]; bk = in_proj_b[H:2 * H]; bv = in_proj_b[2 * H:3 * H]
    c_vec = out_proj_w.sum(axis=0)
    bsum = float(out_proj_b.sum())
    scale = 1.0 / np.sqrt(HD)
    u2 = np.stack([(Wv[h * HD:(h + 1) * HD, :] * c_vec[h * HD:(h + 1) * HD, None]).sum(0)
                   for h in range(NH)], axis=1)      # [256, 4]
    beta = np.stack([(bv[h * HD:(h + 1) * HD] * c_vec[h * HD:(h + 1) * HD]).sum()
                     for h in range(NH)])            # [4]

    KIN = x.shape[1]
    assert KIN == P, "stage A assumes IN_CH == 128"
    meta = dict(NPAD=NPAD, NBLK=NBLK, NBC=NBC, T=T, TE=TE, NDT=NDT, EPC=EPC,
                EPC_raw=EPC_raw, EP=EP, bsum=bsum, KW=KW, invmap=invmap,
                active=tuple(map(tuple, active)))

    common = {
        "dinv_cols": dinv_cols.astype(np.float32),
        "w1": np.asarray(W1, np.float32).astype(bf16),                      # [128,256]
        "w2c": np.asarray(W2, np.float32).reshape(2, P, H).astype(bf16),    # chunks of rows
        "wqc": (np.asarray(Wq, np.float32).T * scale).reshape(2, P, H).astype(bf16),
        "wkc": np.asarray(Wk, np.float32).T.reshape(2, P, H).astype(bf16),
        "uc": u2.reshape(2, P, NH).astype(bf16),
        "b1r": np.asarray(b1, np.float32).reshape(1, H).astype(bf16),
        "b2r": np.asarray(b2, np.float32).reshape(1, H).astype(bf16),
        "bqr": (np.asarray(bq, np.float32) * scale).reshape(1, H).astype(bf16),
        "bkr": np.asarray(bk, np.float32).reshape(1, H).astype(bf16),
        "betar": beta.reshape(1, NH).astype(np.float32),
        "iota_row": np.tile(np.arange(P, dtype=np.float32).astype(bf16)[None, :], (P, 1)),
        "ident_bf": np.eye(P, dtype=np.float32).astype(bf16),
        "ident_f32": np.eye(P, dtype=np.float32),
    }
    in_maps = []
    for c in range(NCORES):
        m = dict(common)
        m["xT"] = np.ascontiguousarray(xT[:, c * NBC * P:(c + 1) * NBC * P])
        m["gsrc"] = gsrc_core[c]
        m["dstloc"] = dstloc_core[c]
        m["dinv_own"] = np.ascontiguousarray(dinv_cols[:, c * NBC:(c + 1) * NBC]).astype(np.float32)
        m["sploc"] = sploc_core[c]
        m["dpidx"] = dpi[c]
        in_maps.append(m)
    return in_maps, meta


# ----------------------------------------------------------------------------
# program builder
# ----------------------------------------------------------------------------
def build_program(meta):
    NPAD, NBLK, NBC, T, TE, NDT, KW = (meta[k] for k in
                                   ("NPAD", "NBLK", "NBC", "T", "TE", "NDT", "KW"))
    H = HIDDEN
    TW = 264  # packed table width

    nc = bacc.Bacc("TRN2", target_bir_lowering=False, debug=False,
                   num_devices=NCORES)

    def din(name, shape, dt):
        return nc.dram_tensor(name, shape, dt, kind="ExternalInput")

    xT = din("xT", [P, NBC * P], BF)
    dinv_cols = din("dinv_cols", [P, NBLK], F32)
    dinv_own = din("dinv_own", [P, NBC], F32)
    w1 = din("w1", [P, H], BF)
    w2c = din("w2c", [2, P, H], BF)
    wqc = din("wqc", [2, P, H], BF)
    wkc = din("wkc", [2, P, H], BF)
    uc = din("uc", [2, P, NH], BF)
    b1r = din("b1r", [1, H], BF)
    b2r = din("b2r", [1, H], BF)
    bqr = din("bqr", [1, H], BF)
    bkr = din("bkr", [1, H], BF)
    betar = din("betar", [1, NH], F32)
    iota_in = din("iota_row", [P, P], BF)
    identb_in = din("ident_bf", [P, P], BF)
    identf_in = din("ident_f32", [P, P], F32)
    gsrc_in = din("gsrc", [P, NBC * T], I32)
    dstloc_in = din("dstloc", [P, NBC * T], BF)
    sploc_in = din("sploc", [P, NDT * KW], BF)
    dpidx_in = din("dpidx", [P, NDT], I32)

    out_t = nc.dram_tensor("out", [NDT * P], F32, kind="ExternalOutput")

    hxd_shard = nc.dram_tensor("hxd_shard", [NBC * P, H], BF, kind="Internal")
    hxd = nc.dram_tensor("hxd", [NPAD, H], BF, kind="Internal", addr_space="Shared")
    hw2d_shard = nc.dram_tensor("hw2d_shard", [NBC * P, H], BF, kind="Internal")
    hw2d_full = nc.dram_tensor("hw2d_full", [NPAD, H], BF, kind="Internal", addr_space="Shared")
    tqk_shard = nc.dram_tensor("tqk_shard", [NBC * P, 2 * TW], BF, kind="Internal")
    tqk_full = nc.dram_tensor("tqk_full", [NPAD, 2 * TW], BF, kind="Internal", addr_space="Shared")

    AG = mybir.AluOpType
    with tile.TileContext(nc) as tc:
        with tc.tile_pool(name="sb", bufs=1) as res, \
             tc.tile_pool(name="wk", bufs=3) as wk, \
             tc.tile_pool(name="gp", bufs=12) as gp, \
             tc.tile_pool(name="ps", bufs=4, space="PSUM") as psp, \
             tc.tile_pool(name="pt", bufs=2, space="PSUM") as ptp:

            # ---------------- residents
            def load(name, src, shape, dt):
                t = res.tile(shape, dt, tag=name)
                nc.sync.dma_start(t[:], src[:])
                return t
            w1_t = load("w1", w1, [P, H], BF)

            def load2(name, src, width, dt):
                # [2, P, width] dram chunks -> [P, 2*width] sbuf
                t = res.tile([P, 2 * width], dt, tag=name)
                for k in range(2):
                    nc.sync.dma_start(t[:, k * width:(k + 1) * width], src[k])
                return t
            w2_t = load2("w2c", w2c, H, BF)
            wq_t = load2("wqc", wqc, H, BF)
            wk_t = load2("wkc", wkc, H, BF)
            uc_t = load2("uc", uc, NH, BF)
            iota_t = load("iota", iota_in, [P, P], BF)
            idb_t = load("idb", identb_in, [P, P], BF)
            idf_t = load("idf", identf_in, [P, P], F32)
            dinvc_t = load("dinvc", dinv_cols, [P, NBLK], F32)
            dinvo_t = load("dinvo", dinv_own, [P, NBC], F32)
            gsrc_t = load("gsrc", gsrc_in, [P, NBC * T], I32)
            dstloc_t = load("dstloc", dstloc_in, [P, NBC * T], BF)
            sploc_t = load("sploc", sploc_in, [P, NDT * KW], BF)
            dpidx_t = load("dpidx", dpidx_in, [P, NDT], I32)
            # biases broadcast to 128 partitions via DMA
            def loadb(name, src):
                t = res.tile([P, H], BF, tag=name)
                nc.sync.dma_start(t[:], src[:].to_broadcast((P, H)))
                return t
            b1_t = loadb("b1", b1r)
            b2_t = loadb("b2", b2r)
            bq_t = loadb("bq", bqr)
            bk_t = loadb("bk", bkr)
            beta_b = res.tile([P, NH], F32, tag="betab")
            nc.sync.dma_start(beta_b[:], betar[:].to_broadcast((P, NH)))

            colbuf = res.tile([P, NDT], F32, tag="colbuf")
            bsum_t = res.tile([P, 1], F32, tag="bsum")
            nc.vector.memset(bsum_t[:], float(meta["bsum"]))

            # ---------------- stage A: hxd = (x @ W1 + b1) * dinv  (own shard only)
            QUAD = 4
            for i0 in range(0, NBC, QUAD):
                nq = min(QUAD, NBC - i0)
                xt = wk.tile([P, QUAD * P], BF, tag="xt")
                nc.sync.dma_start(xt[:, :nq * P], xT[:, i0 * P:(i0 + nq) * P])
                for j in range(nq):
                    i = i0 + j
                    ps = psp.tile([P, H], F32, tag="p256", space="PSUM")
                    nc.tensor.matmul(ps[:], lhsT=xt[:, j * P:(j + 1) * P], rhs=w1_t[:],
                                     start=True, stop=True)
                    tmp = wk.tile([P, H], F32, tag="tmpA")
                    nc.vector.tensor_tensor(out=tmp[:], in0=ps[:], in1=b1_t[:], op=AG.add)
                    hx = wk.tile([P, H], BF, tag="hx")
                    nc.scalar.activation(hx[:], tmp[:], mybir.ActivationFunctionType.Copy,
                                         scale=dinvo_t[:, i:i + 1])
                    nc.sync.dma_start(hxd_shard[i * P:(i + 1) * P, :], hx[:])
            nc.gpsimd.collective_compute(
                "AllGather", AG.bypass, replica_groups=[list(range(NCORES))],
                ins=[hxd_shard[:]], outs=[hxd[:]])

            # ---------------- aggregation layer template
            def agg_layer(table, shard, b, finalize):
                agg = psp.tile([P, H], F32, tag="p256", space="PSUM")
                for t in range(T):
                    col = b * T + t
                    g = gp.tile([P, H], BF, tag="g")
                    nc.gpsimd.indirect_dma_start(
                        out=g[:], out_offset=None, in_=table[:],
                        in_offset=bass.IndirectOffsetOnAxis(ap=gsrc_t[:, col:col + 1], axis=0))
                    st = gp.tile([P, P], BF, tag="st")
                    nc.vector.tensor_tensor(
                        out=st[:], in0=iota_t[:],
                        in1=dstloc_t[:, col:col + 1].to_broadcast((P, P)), op=AG.is_equal)
                    nc.tensor.matmul(agg[:], lhsT=st[:], rhs=g[:],
                                     start=(t == 0), stop=(t == T - 1))
                selfb = wk.tile([P, H], BF, tag="selfb")
                nc.sync.dma_start(selfb[:], shard[b * P:(b + 1) * P, :])
                asum = wk.tile([P, H], F32, tag="asum")
                nc.vector.tensor_tensor(out=asum[:], in0=agg[:], in1=selfb[:], op=AG.add)
                finalize(asum)

            def transposed_chunks(src_bf, tag):
                outs = []
                for k in range(2):
                    pt = ptp.tile([P, P], BF, tag="pT", space="PSUM")
                    nc.tensor.transpose(pt[:], src_bf[:, k * P:(k + 1) * P], idb_t[:])
                    sb = wk.tile([P, P], BF, tag=f"{tag}{k}")
                    nc.vector.tensor_copy(out=sb[:], in_=pt[:])
                    outs.append(sb)
                return outs

            # ---------------- layer 1 + transform
            for b in range(NBC):
                def fin1(agg, b=b):
                    h1 = wk.tile([P, H], BF, tag="h1")
                    nc.scalar.activation(h1[:], agg[:], mybir.ActivationFunctionType.Relu,
                                         scale=dinvo_t[:, b:b + 1])
                    hts = transposed_chunks(h1, "h1T")
                    ps2 = psp.tile([P, H], F32, tag="p256", space="PSUM")
                    for k in range(2):
                        nc.tensor.matmul(ps2[:], lhsT=hts[k][:], rhs=w2_t[:, k * H:(k + 1) * H],
                                         start=(k == 0), stop=(k == 1))
                    t2 = wk.tile([P, H], F32, tag="t2")
                    nc.vector.tensor_tensor(out=t2[:], in0=ps2[:], in1=b2_t[:], op=AG.add)
                    hwb = wk.tile([P, H], BF, tag="hwb")
                    nc.scalar.activation(hwb[:], t2[:], mybir.ActivationFunctionType.Copy,
                                         scale=dinvo_t[:, b:b + 1])
                    nc.sync.dma_start(hw2d_shard[b * P:(b + 1) * P, :], hwb[:])
                agg_layer(hxd, hxd_shard, b, fin1)

            nc.gpsimd.collective_compute(
                "AllGather", AG.bypass, replica_groups=[list(range(NCORES))],
                ins=[hw2d_shard[:]], outs=[hw2d_full[:]])

            # ---------------- layer 2 + decode tables
            for b in range(NBC):
                def fin2(agg, b=b):
                    zb = wk.tile([P, H], BF, tag="zb")
                    nc.scalar.activation(zb[:], agg[:], mybir.ActivationFunctionType.Copy,
                                         scale=dinvo_t[:, b:b + 1])
                    zts = transposed_chunks(zb, "zT")
                    tqkb = wk.tile([P, 2 * TW], BF, tag="tqkb")
                    tqb = tqkb[:, 0:TW]
                    tkb = tqkb[:, TW:2 * TW]
                    # Q' = z@WqT*s + bq'
                    psq = psp.tile([P, H], F32, tag="p256", space="PSUM")
                    for k in range(2):
                        nc.tensor.matmul(psq[:], lhsT=zts[k][:], rhs=wq_t[:, k * H:(k + 1) * H],
                                         start=(k == 0), stop=(k == 1))
                    nc.vector.tensor_tensor(out=tqb[:, 0:H], in0=psq[:], in1=bq_t[:], op=AG.add)
                    # K = z@WkT + bk
                    psk = psp.tile([P, H], F32, tag="p256", space="PSUM")
                    for k in range(2):
                        nc.tensor.matmul(psk[:], lhsT=zts[k][:], rhs=wk_t[:, k * H:(k + 1) * H],
                                         start=(k == 0), stop=(k == 1))
                    nc.vector.tensor_tensor(out=tkb[:, 0:H], in0=psk[:], in1=bk_t[:], op=AG.add)
                    # l0 per head
                    qk = wk.tile([P, H], F32, tag="qk")
                    nc.vector.tensor_tensor(out=qk[:], in0=tqb[:, 0:H], in1=tkb[:, 0:H], op=AG.mult)
                    l0 = wk.tile([P, NH], F32, tag="l0")
                    nc.vector.tensor_reduce(out=l0[:], in_=qk[:].rearrange("p (h d) -> p h d", h=NH),
                                            axis=mybir.AxisListType.X, op=AG.add)
                    nc.vector.tensor_copy(out=tqb[:, H:H + NH], in_=l0[:])
                    # S per head
                    pss = ptp.tile([P, NH], F32, tag="pS", space="PSUM")
                    for k in range(2):
                        nc.tensor.matmul(pss[:], lhsT=zts[k][:], rhs=uc_t[:, k * NH:(k + 1) * NH],
                                         start=(k == 0), stop=(k == 1))
                    sf = wk.tile([P, NH], F32, tag="sf")
                    nc.vector.tensor_tensor(out=sf[:], in0=pss[:], in1=beta_b[:], op=AG.add)
                    nc.vector.tensor_copy(out=tqb[:, H + NH:H + 2 * NH], in_=sf[:])
                    nc.vector.tensor_copy(out=tkb[:, H:H + NH], in_=sf[:])
                    nc.vector.memset(tkb[:, H + NH:TW], 0)
                    nc.sync.dma_start(tqk_shard[b * P:(b + 1) * P, :], tqkb[:])
                agg_layer(hw2d_full, hw2d_shard, b, fin2)

            nc.gpsimd.collective_compute(
                "AllGather", AG.bypass, replica_groups=[list(range(NCORES))],
                ins=[tqk_shard[:]], outs=[tqk_full[:]])

            # ---------------- decode (Q side streamed from panels, K side gathered)
            DG = 4  # tiles per vector batch
            assert NDT % DG == 0
            W = KW + 2
            panelbuf = res.tile([P, W * TW], BF, tag="panelbuf")
            next_p = 0
            for g0 in range(0, NDT, DG):
                gq = wk.tile([P, DG, TW], BF, tag="gq")
                gk = gp.tile([P, DG, TW], BF, tag="gk")
                for j in range(DG):
                    t = g0 + j
                    phi_t = (t * NBLK) // NDT
                    while next_p < min(phi_t + KW, NBLK):
                        nc.sync.dma_start(
                            panelbuf[:, (next_p % W) * TW:(next_p % W + 1) * TW],
                            tqk_full[next_p * P:(next_p + 1) * P, 0:TW])
                        next_p += 1
                    nc.gpsimd.indirect_dma_start(
                        out=gk[:, j, :], out_offset=None, in_=tqk_full[:],
                        in_offset=bass.IndirectOffsetOnAxis(ap=dpidx_t[:, t:t + 1], axis=0),
                        element_offset=TW)
                    psq = psp.tile([P, TW], F32, tag="p256", space="PSUM")
                    ks = [k for k in range(KW)
                          if phi_t + k < NBLK and meta["active"][t][k]]
                    if not ks:
                        ks = [0]
                    for ki, k in enumerate(ks):
                        p = phi_t + k
                        rt = gp.tile([P, P], BF, tag="rt")
                        nc.vector.tensor_tensor(
                            out=rt[:], in0=iota_t[:],
                            in1=sploc_t[:, t * KW + k:t * KW + k + 1].to_broadcast((P, P)),
                            op=AG.is_equal)
                        prt = ptp.tile([P, P], BF, tag="pT", space="PSUM")
                        nc.tensor.transpose(prt[:], rt[:], idb_t[:])
                        Rb = gp.tile([P, P], BF, tag="Rb")
                        nc.vector.tensor_copy(out=Rb[:], in_=prt[:])
                        nc.tensor.matmul(psq[:], lhsT=Rb[:],
                                         rhs=panelbuf[:, (p % W) * TW:(p % W) * TW + TW],
                                         start=(ki == 0), stop=(ki == len(ks) - 1))
                    nc.vector.tensor_copy(out=gq[:, j, :], in_=psq[:])
                prod = wk.tile([P, DG, H], F32, tag="prod")
                nc.vector.tensor_tensor(out=prod[:], in0=gq[:, :, 0:H], in1=gk[:, :, 0:H], op=AG.mult)
                l1 = wk.tile([P, DG * NH], F32, tag="l1")
                nc.vector.tensor_reduce(out=l1[:], in_=prod[:].rearrange("p g (h d) -> p (g h) d", h=NH),
                                        axis=mybir.AxisListType.X, op=AG.add)
                dlt = wk.tile([P, DG * NH], F32, tag="dlt")
                nc.vector.tensor_tensor(out=dlt[:].rearrange("p (g h) -> p g h", h=NH),
                                        in0=l1[:].rearrange("p (g h) -> p g h", h=NH),
                                        in1=gq[:, :, H:H + NH], op=AG.subtract)
                a1 = wk.tile([P, DG * NH], F32, tag="a1")
                nc.scalar.activation(a1[:], dlt[:], mybir.ActivationFunctionType.Sigmoid)
                ds = wk.tile([P, DG * NH], F32, tag="ds")
                nc.vector.tensor_tensor(out=ds[:].rearrange("p (g h) -> p g h", h=NH),
                                        in0=gk[:, :, H:H + NH],
                                        in1=gq[:, :, H + NH:H + 2 * NH],
                                        op=AG.subtract)
                pr = wk.tile([P, DG * NH], F32, tag="pr")
                nc.vector.tensor_tensor(out=pr[:], in0=a1[:], in1=ds[:], op=AG.mult)
                prs = wk.tile([P, DG], F32, tag="prs")
                nc.vector.tensor_reduce(out=prs[:], in_=pr[:].rearrange("p (g h) -> p g h", h=NH),
                                        axis=mybir.AxisListType.X, op=AG.add)
                s0s = wk.tile([P, DG], F32, tag="s0s")
                nc.vector.tensor_reduce(out=s0s[:], in_=gq[:, :, H + NH:H + 2 * NH],
                                        axis=mybir.AxisListType.X, op=AG.add)
                rr = wk.tile([P, DG], F32, tag="rr")
                nc.vector.tensor_tensor(out=rr[:], in0=prs[:], in1=s0s[:], op=AG.add)
                nc.scalar.activation(colbuf[:, g0:g0 + DG], rr[:],
                                     mybir.ActivationFunctionType.Sigmoid, bias=bsum_t[:])

            # transpose colbuf -> out
            for c0 in range(0, NDT, P):
                w = min(P, NDT - c0)
                po = ptp.tile([P, P], F32, tag="pT", space="PSUM")
                nc.tensor.transpose(po[:w, :], colbuf[:, c0:c0 + w], idf_t[:])
                ob = wk.tile([P, P], F32, tag="ob")
                nc.vector.tensor_copy(out=ob[:w, :], in_=po[:w, :])
                nc.sync.dma_start(
                    out_t[c0 * P:(c0 + w) * P].rearrange("(a b) -> a b", b=P), ob[:w, :])
    nc.compile()
    return nc


# ----------------------------------------------------------------------------
_CACHE = {}


TRACE = False
LAST_EXEC_NS = None


def kernel(**inputs):
    import concourse.bass_utils as bass_utils
    global LAST_EXEC_NS
    in_maps, meta = build_host_data(**inputs)
    key = (meta["NPAD"], meta["NBLK"], meta["T"], meta["NDT"], hash(meta["active"]))
    if key not in _CACHE:
        _CACHE[key] = build_program(meta)
    nc = _CACHE[key]
    trace = bool(TRACE)
    if trace:
        try:
            from trn_agent_boot.trn_boot import _ntff_profile_via_ctypes
            import antenv.axon_hooks as ah
            if ah.get_axon_ntff_profile_hook() is None:
                ah.set_axon_ntff_profile_hook(
                    _ntff_profile_via_ctypes("/opt/axon/libaxon_pjrt.so"))
        except Exception:
            trace = False
    res = bass_utils.run_bass_kernel_spmd(nc, in_maps, core_ids=list(range(NCORES)),
                                          trace=trace)
    LAST_EXEC_NS = res.exec_time_ns
    EP = meta["EP"]
    out = np.zeros(EP, np.float32)
    for c in range(NCORES):
        inv = meta["invmap"][c]
        m = inv >= 0
        out[inv[m]] = res.results[c]["out"][m]
    return out

